# revision 11
# baseline (speedup 1.0000x reference)
"""Trainium2 Bass kernel for nn_DecisionActionAuxiliaryHeads.

Distribution (8 NeuronCores, tensor-parallel):
  - W1/W2 column-sharded (256 cols/core); hidden activations AllGather'd
    between the two MLP layers (transposed layout so no on-device transposes).
  - Warg tensor-sharded over the argument dim (16 args/core). Wname is folded
    into every Warg column on the host (softmax shift-invariance makes the
    pollution cancel in the arg head), so one fused matmul produces
    name+arg+bias scores for all 64*16 (name, local-arg) columns per core.
  - Candidate / target-arg / name-head values are extracted with segmented
    masked reductions on the Vector engine and scattered into a per-core
    partial tensor, which a ReduceScatter sums and splits over the batch.
  - Loss tail (masked log-softmax NLLs), score masking and argmax run
    per-core on the 16 owned batch rows.

Host side only shards/stages data (slicing, transposes, one-hot/index
encoding); all FLOPs of the model run on the NeuronCores.
"""

import numpy as np

N_CORES = 8
B, S, H, P = 128, 512, 2048, 2048
NN, NA, C = 64, 128, 512
BS = B // N_CORES          # batch rows per core after ReduceScatter
ALS = NA // N_CORES        # args per core
JCOLS = NN * ALS           # 1024 fused (name, local-arg) columns per core
SEG = 8                    # segment width for the candidate gather
NSEG = JCOLS // SEG        # 128
LOGIT_FLOOR = -1e9

# S (partial-sum tensor) column layout: [scores C | arg head NA | name head NN]
SCOL_ARG = C
SCOL_NAME = C + NA
SWIDTH = C + NA + NN       # 704

_CACHE = {}


def _build(Q, stage=10):
    import concourse.bacc as bacc
    import concourse.tile as tile
    import concourse.mybir as mybir

    f32 = mybir.dt.float32
    bf16 = mybir.dt.bfloat16
    i16 = mybir.dt.int16
    u16 = mybir.dt.uint16
    u32 = mybir.dt.uint32
    u8 = mybir.dt.uint8
    Alu = mybir.AluOpType
    Act = mybir.ActivationFunctionType

    GW = NSEG * Q * SEG            # candidate product elements per row
    NG = NSEG * Q                  # candidate gather slots per row
    GTOT = NG + ALS + N_CORES      # + arg-head block + name-head block

    nc = bacc.Bacc("TRN2", target_bir_lowering=False, debug=False,
                   num_devices=N_CORES)

    din = lambda name, shape, dt: nc.dram_tensor(name, shape, dt, kind="ExternalInput")
    dout = lambda name, shape, dt: nc.dram_tensor(name, shape, dt, kind="ExternalOutput")

    pooledT = din("pooledT", [P, B], f32)
    w1 = din("w1", [H, P // N_CORES], f32)
    w2 = din("w2", [P, P // N_CORES], f32)
    waug = din("waug", [P + 1, JCOLS + N_CORES], f32)
    adaptT = din("adaptT", [P // N_CORES, B], f32)
    ascale = din("ascale", [B, 1], f32)
    cmask = din("cmask", [B, GW], bf16)
    sidx = din("sidx", [B, 2 * GTOT], i16)
    ohtid = din("ohtid", [B, NN], f32)
    meff_s = din("meff_s", [BS, C], f32)
    floor_s = din("floor_s", [BS, C], f32)
    meff_a = din("meff_a", [BS, NA], f32)
    floor_a = din("floor_a", [BS, NA], f32)
    meff_n = din("meff_n", [BS, NN], f32)
    floor_n = din("floor_n", [BS, NN], f32)
    oh_ta = din("oh_ta", [BS, NA], f32)
    oh_tn = din("oh_tn", [BS, NN], f32)
    iota_c = din("iota_c", [BS, C], f32)

    out_scores = dout("out_scores", [BS, C], f32)
    out_pred = dout("out_pred", [BS, 1], u32)
    out_nll = dout("out_nll", [BS, 2], f32)

    KT = H // 128  # 16 contraction tiles
    MSH = P // N_CORES  # 256

    with tile.TileContext(nc) as tc:
        with (
            tc.tile_pool(name="wk", bufs=4) as wk,
            tc.tile_pool(name="acts", bufs=2) as acts,
            tc.tile_pool(name="persist", bufs=1) as persist,
            tc.tile_pool(name="psum", bufs=2, space="PSUM") as psum,
            tc.tile_pool(name="psbig", bufs=1, space="PSUM") as psbig,
            tc.tile_pool(name="dram", bufs=1, space="DRAM") as dram,
            tc.tile_pool(name="tail", bufs=1) as tailp,
        ):
            # ---- preload pooledT k-tiles (reused by both m-tiles) ----
            pt = []
            for kt in range(KT):
                t = persist.tile([128, B], f32, tag=f"pt{kt}")
                nc.sync.dma_start(t[:], pooledT[kt * 128:(kt + 1) * 128, :])
                pt.append(t)

            # ---- MLP layer 1: h1T shard = silu(W1_sh^T @ pooled) ----
            h1_bounce = dram.tile([MSH, B], f32)
            for mt in range(2):
                ps = psum.tile([128, B], f32, space="PSUM", tag="zps")
                for kt in range(KT):
                    lw = wk.tile([128, 128], f32, tag="w1t")
                    nc.sync.dma_start(
                        lw[:], w1[kt * 128:(kt + 1) * 128, mt * 128:(mt + 1) * 128])
                    nc.tensor.matmul(ps[:], lw[:], pt[kt][:],
                                     start=(kt == 0), stop=(kt == KT - 1))
                sg = acts.tile([128, B], f32, tag="h1sg")
                nc.scalar.activation(sg[:], ps[:], Act.Sigmoid)
                h = acts.tile([128, B], f32, tag="h1t")
                nc.vector.tensor_tensor(h[:], ps[:], sg[:], op=Alu.mult)
                nc.sync.dma_start(h1_bounce[mt * 128:(mt + 1) * 128, :], h[:])

            ag_h1 = dram.tile([P, B], f32)
            nc.gpsimd.collective_compute(
                "AllGather", Alu.bypass,
                replica_groups=[list(range(N_CORES))],
                ins=[h1_bounce.opt()], outs=[ag_h1.opt()])

            # ---- MLP layer 2: featsT shard = silu(W2_sh^T @ h1) + adapter ----
            h1t = []
            for kt in range(KT):
                t = persist.tile([128, B], f32, tag=f"h1{kt}")
                nc.sync.dma_start(t[:], ag_h1[kt * 128:(kt + 1) * 128, :])
                h1t.append(t)

            asc = persist.tile([B, 1], f32, tag="ascale")
            nc.sync.dma_start(asc[:], ascale[:, :])

            f_bounce = dram.tile([MSH, B], f32)
            for mt in range(2):
                ps = psum.tile([128, B], f32, space="PSUM", tag="zps")
                for kt in range(KT):
                    lw = wk.tile([128, 128], f32, tag="w2t")
                    nc.sync.dma_start(
                        lw[:], w2[kt * 128:(kt + 1) * 128, mt * 128:(mt + 1) * 128])
                    nc.tensor.matmul(ps[:], lw[:], h1t[kt][:],
                                     start=(kt == 0), stop=(kt == KT - 1))
                fsg = acts.tile([128, B], f32, tag="fsg")
                nc.scalar.activation(fsg[:], ps[:], Act.Sigmoid)
                f = acts.tile([128, B], f32, tag="ft")
                nc.vector.tensor_tensor(f[:], ps[:], fsg[:], op=Alu.mult)
                at = acts.tile([128, B], f32, tag="adt")
                nc.sync.dma_start(at[:], adaptT[mt * 128:(mt + 1) * 128, :])
                f2 = acts.tile([128, B], f32, tag="ft2")
                nc.vector.scalar_tensor_tensor(
                    f2[:], at[:], asc[:], f[:], Alu.mult, Alu.add)
                nc.sync.dma_start(f_bounce[mt * 128:(mt + 1) * 128, :], f2[:])

            ag_f = dram.tile([P, B], f32)
            nc.gpsimd.collective_compute(
                "AllGather", Alu.bypass,
                replica_groups=[list(range(N_CORES))],
                ins=[f_bounce.opt()], outs=[ag_f.opt()])

            if stage <= 1:
                dbg = tailp.tile([BS, B], f32, tag="dbg")
                nc.sync.dma_start(dbg[:], ag_f[0:BS, 0:B])
                nc.sync.dma_start(out_scores[:, 0:B], dbg[:])

            if stage >= 2:
                # ---- fused head matmul ALLB = [feats,1] @ waug ----
                NW = JCOLS + N_CORES  # 1032
                pA = psbig.tile([128, 512], f32, space="PSUM", tag="pA")
                pB = psbig.tile([128, 512], f32, space="PSUM", tag="pB")
                pN = psbig.tile([128, N_CORES], f32, space="PSUM", tag="pN")
                ones1 = persist.tile([1, B], f32, tag="ones1")
                nc.vector.memset(ones1[:], 1.0)
                for kt in range(KT + 1):
                    if kt < KT:
                        lf = wk.tile([128, B], f32, tag="ft_k")
                        nc.sync.dma_start(lf[:], ag_f[kt * 128:(kt + 1) * 128, :])
                        rw = wk.tile([128, NW], f32, tag="waug_k")
                        nc.sync.dma_start(rw[:], waug[kt * 128:(kt + 1) * 128, :])
                        lhsT, rhs = lf, rw
                    else:
                        rb = wk.tile([1, NW], f32, tag="waug_b")
                        nc.sync.dma_start(rb[:], waug[P:P + 1, :])
                        lhsT, rhs = ones1, rb
                    st, sp = (kt == 0), (kt == KT)
                    nc.tensor.matmul(pA[:], lhsT[:], rhs[:, 0:512], start=st, stop=sp)
                    nc.tensor.matmul(pB[:], lhsT[:], rhs[:, 512:1024], start=st, stop=sp)
                    nc.tensor.matmul(pN[:], lhsT[:], rhs[:, 1024:NW], start=st, stop=sp)

                allb = persist.tile([B, JCOLS], f32, tag="allb")
                nc.vector.tensor_copy(allb[:, 0:512], pA[:])
                nc.vector.tensor_copy(allb[:, 512:1024], pB[:])

                if stage == 2:
                    nc.sync.dma_start(out_scores[:, :], allb[0:BS, 0:C])

            if stage >= 3:
                # ---- gathers ----
                gath = persist.tile([B, GTOT], f32, tag="gath")

                cm = persist.tile([B, GW], bf16, tag="cmask")
                nc.sync.dma_start(cm[:], cmask[:, :])
                prod = persist.tile([B, GW], f32, tag="prod")
                allb_v = (allb[:].rearrange("p (s t) -> p s t", t=SEG)
                          .unsqueeze(2).to_broadcast([B, NSEG, Q, SEG]))
                cm_v = cm[:].rearrange("p (s q t) -> p s q t", q=Q, t=SEG)
                prod_v = prod[:].rearrange("p (s q t) -> p s q t", q=Q, t=SEG)
                nc.vector.tensor_tensor(prod_v, allb_v, cm_v, op=Alu.mult)
                nc.vector.tensor_reduce(
                    gath[:, 0:NG],
                    prod[:].rearrange("p (g t) -> p g t", t=SEG),
                    axis=mybir.AxisListType.X, op=Alu.add)

                oht = persist.tile([B, NN], f32, tag="ohtid")
                nc.sync.dma_start(oht[:], ohtid[:, :])
                aprod = persist.tile([B, ALS * NN], f32, tag="aprod")
                allb_a = allb[:].rearrange("p (n q) -> p q n", q=ALS)
                oht_v = oht[:].unsqueeze(1).to_broadcast([B, ALS, NN])
                aprod_v = aprod[:].rearrange("p (q n) -> p q n", n=NN)
                nc.vector.tensor_tensor(aprod_v, allb_a, oht_v, op=Alu.mult)
                nc.vector.tensor_reduce(
                    gath[:, NG:NG + ALS],
                    aprod[:].rearrange("p (q n) -> p q n", n=NN),
                    axis=mybir.AxisListType.X, op=Alu.add)

                nc.vector.tensor_copy(gath[:, NG + ALS:GTOT], pN[:])

                if stage == 3:
                    nc.sync.dma_start(out_scores[:, :], gath[0:BS, 0:C])

            if stage >= 4:
                # ---- scatter into partial-sum tensor ----
                stile = persist.tile([B, SWIDTH], f32, tag="stile")
                nc.vector.memset(stile[:], 0.0)
                sx = persist.tile([B, 2 * GTOT], i16, tag="sidx")
                nc.sync.dma_start(sx[:], sidx[:, :])
                nc.gpsimd.local_scatter(
                    stile[:].bitcast(u16), gath[:].bitcast(u16), sx[:],
                    channels=B, num_elems=2 * SWIDTH, num_idxs=2 * GTOT)

                if stage == 4:
                    nc.sync.dma_start(out_scores[:, :], stile[0:BS, 0:C])

            if stage >= 5:
                # ---- ReduceScatter over cores / batch rows ----
                s_bounce = dram.tile([B, SWIDTH], f32)
                nc.sync.dma_start(s_bounce[:], stile[:])
                rs_out = dram.tile([BS, SWIDTH], f32)
                nc.gpsimd.collective_compute(
                    "ReduceScatter", Alu.add,
                    replica_groups=[list(range(N_CORES))],
                    ins=[s_bounce.opt()], outs=[rs_out.opt()])
                srs = tailp.tile([BS, SWIDTH], f32, tag="srs")
                nc.sync.dma_start(srs[:], rs_out[:, :])

                if stage == 5:
                    nc.sync.dma_start(out_scores[:, :], srs[:, 0:C])

            if stage >= 6:
                # ---- tails on the 16 owned rows ----
                def load(src, shape, tag):
                    t = tailp.tile(shape, f32, tag=tag)
                    nc.sync.dma_start(t[:], src[:, :])
                    return t

                ms = load(meff_s, [BS, C], "meff_s")
                fs = load(floor_s, [BS, C], "floor_s")
                sc_m = tailp.tile([BS, C], f32, tag="sc_m")
                nc.vector.tensor_tensor(sc_m[:], srs[:, 0:C], ms[:], op=Alu.mult)
                nc.vector.tensor_tensor(sc_m[:], sc_m[:], fs[:], op=Alu.add)
                nc.sync.dma_start(out_scores[:, :], sc_m[:])

            if stage >= 7:
                # preds = first index of max
                mx = tailp.tile([BS, 1], f32, tag="mx")
                nc.vector.tensor_reduce(mx[:], sc_m[:], axis=mybir.AxisListType.X,
                                        op=Alu.max)
                eq = tailp.tile([BS, C], u8, tag="eq")
                nc.vector.tensor_tensor(eq[:], sc_m[:], mx[:].to_broadcast([BS, C]),
                                        op=Alu.is_equal)
                io = load(iota_c, [BS, C], "iota_c")
                idxm = tailp.tile([BS, C], f32, tag="idxm")
                nc.vector.memset(idxm[:], 1.0e9)
                nc.vector.copy_predicated(idxm[:], eq[:], io[:])
                predf = tailp.tile([BS, 1], f32, tag="predf")
                nc.vector.tensor_reduce(predf[:], idxm[:], axis=mybir.AxisListType.X,
                                        op=Alu.min)
                predu = tailp.tile([BS, 1], u32, tag="predu")
                nc.vector.tensor_copy(predu[:], predf[:])
                nc.sync.dma_start(out_pred[:, :], predu[:])

            if stage >= 8:
                # masked log-softmax NLL for both heads
                nll = tailp.tile([BS, 2], f32, tag="nll")

                def head(x_ap, W, meff_t, floor_t, oh_t, col, tag):
                    xm = tailp.tile([BS, W], f32, tag=f"xm{tag}")
                    nc.vector.tensor_tensor(xm[:], x_ap, meff_t[:], op=Alu.mult)
                    nc.vector.tensor_tensor(xm[:], xm[:], floor_t[:], op=Alu.add)
                    m1 = tailp.tile([BS, 1], f32, tag=f"m1{tag}")
                    nc.vector.tensor_reduce(m1[:], xm[:], axis=mybir.AxisListType.X,
                                            op=Alu.max)
                    negm = tailp.tile([BS, 1], f32, tag=f"nm{tag}")
                    nc.scalar.mul(negm[:], m1[:], -1.0)
                    e = tailp.tile([BS, W], f32, tag=f"e{tag}")
                    nc.scalar.activation(e[:], xm[:], Act.Exp, bias=negm[:])
                    s1 = tailp.tile([BS, 1], f32, tag=f"s1{tag}")
                    nc.vector.tensor_reduce(s1[:], e[:], axis=mybir.AxisListType.X,
                                            op=Alu.add)
                    if stage <= 8:
                        nc.vector.tensor_copy(nll[:, col:col + 1], s1[:])
                        return
                    ls = tailp.tile([BS, 1], f32, tag=f"ls{tag}")
                    nc.scalar.activation(ls[:], s1[:], Act.Ln)
                    lse = tailp.tile([BS, 1], f32, tag=f"lse{tag}")
                    nc.vector.tensor_tensor(lse[:], m1[:], ls[:], op=Alu.add)
                    if stage <= 9:
                        nc.vector.tensor_copy(nll[:, col:col + 1], lse[:])
                        return
                    scr = tailp.tile([BS, W], f32, tag=f"scr{tag}")
                    nc.vector.tensor_tensor(scr[:], xm[:], oh_t[:], op=Alu.mult)
                    xt = tailp.tile([BS, 1], f32, tag=f"xt{tag}")
                    nc.vector.tensor_reduce(xt[:], scr[:], axis=mybir.AxisListType.X,
                                            op=Alu.add)
                    nc.vector.tensor_tensor(nll[:, col:col + 1], lse[:], xt[:],
                                            op=Alu.subtract)

                ma = load(meff_a, [BS, NA], "meff_a")
                fa = load(floor_a, [BS, NA], "floor_a")
                ota = load(oh_ta, [BS, NA], "oh_ta")
                mn = load(meff_n, [BS, NN], "meff_n")
                fn = load(floor_n, [BS, NN], "floor_n")
                otn = load(oh_tn, [BS, NN], "oh_tn")
                head(srs[:, SCOL_NAME:SCOL_NAME + NN], NN, mn, fn, otn, 0, "n")
                head(srs[:, SCOL_ARG:SCOL_ARG + NA], NA, ma, fa, ota, 1, "a")
                nc.sync.dma_start(out_nll[:, :], nll[:])

    nc.compile()
    return nc


def _effective_mask(m):
    m = m.astype(bool)
    any_row = m.any(axis=1, keepdims=True)
    return np.where(any_row, m, True).astype(np.float32)


def _host_prep(inputs):
    hs = np.asarray(inputs["hidden_states"], np.float32)
    am = np.asarray(inputs["attention_mask"])
    lengths = np.clip(am.astype(np.int64).sum(1), 1, None) - 1
    pooled = hs[np.arange(hs.shape[0]), lengths]
    pooledT = np.ascontiguousarray(pooled.T)

    W1 = np.asarray(inputs["W1"], np.float32)
    W2 = np.asarray(inputs["W2"], np.float32)
    adapter_emb = np.asarray(inputs["adapter_emb"], np.float32)
    adapter_scale = np.float32(np.asarray(inputs["adapter_scale"]))
    Wname = np.asarray(inputs["Wname"], np.float32)
    bname = np.asarray(inputs["bname"], np.float32)
    Warg = np.asarray(inputs["Warg"], np.float32)
    barg = np.asarray(inputs["barg"], np.float32)

    adapt = adapter_emb[np.asarray(inputs["adapter_ids"]).astype(np.int64)]
    adaptT = np.ascontiguousarray(adapt.T)  # [P, B]

    cn = np.asarray(inputs["full_candidate_name_ids"]).astype(np.int64)
    ca = np.asarray(inputs["full_candidate_argument_ids"]).astype(np.int64)
    fcm = np.asarray(inputs["full_candidate_masks"])
    tid = np.asarray(inputs["target_name_ids"]).astype(np.int64)
    ta = np.asarray(inputs["target_argument_ids"]).astype(np.int64)
    ncm = np.asarray(inputs["name_candidate_masks"])
    acm = np.asarray(inputs["target_argument_candidate_masks"])

    # global Q: max candidates per (core, row, seg)
    Q = 1
    per_core = []
    for r in range(N_CORES):
        own = (ca >= ALS * r) & (ca < ALS * (r + 1))
        j = cn * ALS + (ca - ALS * r)  # valid where own
        per_core.append((own, j))
        for b in range(B):
            js = j[b][own[b]]
            if len(js):
                Q = max(Q, int(np.bincount(js // SEG, minlength=NSEG).max()))
    Q = max(Q, 6)

    NG = NSEG * Q
    GTOT = NG + ALS + N_CORES

    meff_s = _effective_mask(fcm)
    floor_s = (1.0 - meff_s) * np.float32(LOGIT_FLOOR)
    meff_n = _effective_mask(ncm)
    floor_n = (1.0 - meff_n) * np.float32(LOGIT_FLOOR)
    meff_a = _effective_mask(acm)
    floor_a = (1.0 - meff_a) * np.float32(LOGIT_FLOOR)
    oh_tn = np.zeros((B, NN), np.float32)
    oh_tn[np.arange(B), tid] = 1.0
    oh_ta = np.zeros((B, NA), np.float32)
    oh_ta[np.arange(B), ta] = 1.0
    ohtid = oh_tn
    iota_c = np.broadcast_to(np.arange(C, dtype=np.float32), (B, C)).copy()

    import ml_dtypes
    bf16 = ml_dtypes.bfloat16

    MS = P // N_CORES
    in_maps = []
    for r in range(N_CORES):
        own, j = per_core[r]
        cmask = np.zeros((B, NSEG, Q, SEG), np.float32)
        sidx = np.full((B, 2 * GTOT), -1, np.int16)
        for b in range(B):
            cnt = np.zeros(NSEG, np.int64)
            for c in np.flatnonzero(own[b]):
                jj = int(j[b, c])
                s, t = jj // SEG, jj % SEG
                q = cnt[s]
                cnt[s] = q + 1
                cmask[b, s, q, t] = 1.0
                g = s * Q + q
                sidx[b, 2 * g] = 2 * c
                sidx[b, 2 * g + 1] = 2 * c + 1
            for q in range(ALS):
                g = NG + q
                d = SCOL_ARG + ALS * r + q
                sidx[b, 2 * g] = 2 * d
                sidx[b, 2 * g + 1] = 2 * d + 1
            for q in range(N_CORES):
                g = NG + ALS + q
                d = SCOL_NAME + N_CORES * r + q
                sidx[b, 2 * g] = 2 * d
                sidx[b, 2 * g + 1] = 2 * d + 1

        wall = np.ascontiguousarray(
            Warg[:, :, ALS * r:ALS * (r + 1)].transpose(1, 0, 2))  # [P, NN, ALS]
        wall = wall + Wname[:, :, None]
        biasrow = (barg[:, ALS * r:ALS * (r + 1)] + bname[:, None]).reshape(1, -1)
        waug = np.empty((P + 1, JCOLS + N_CORES), np.float32)
        waug[:P, :JCOLS] = wall.reshape(P, JCOLS)
        waug[:P, JCOLS:] = Wname[:, N_CORES * r:N_CORES * (r + 1)]
        waug[P, :JCOLS] = biasrow
        waug[P, JCOLS:] = bname[N_CORES * r:N_CORES * (r + 1)]

        rows = slice(BS * r, BS * (r + 1))
        in_maps.append({
            "pooledT": pooledT,
            "w1": np.ascontiguousarray(W1[:, MS * r:MS * (r + 1)]),
            "w2": np.ascontiguousarray(W2[:, MS * r:MS * (r + 1)]),
            "waug": waug,
            "adaptT": np.ascontiguousarray(adaptT[MS * r:MS * (r + 1), :]),
            "ascale": np.full((B, 1), adapter_scale, np.float32),
            "cmask": cmask.reshape(B, -1).astype(bf16),
            "sidx": sidx,
            "ohtid": ohtid,
            "meff_s": np.ascontiguousarray(meff_s[rows]),
            "floor_s": np.ascontiguousarray(floor_s[rows]),
            "meff_a": np.ascontiguousarray(meff_a[rows]),
            "floor_a": np.ascontiguousarray(floor_a[rows]),
            "meff_n": np.ascontiguousarray(meff_n[rows]),
            "floor_n": np.ascontiguousarray(floor_n[rows]),
            "oh_ta": np.ascontiguousarray(oh_ta[rows]),
            "oh_tn": np.ascontiguousarray(oh_tn[rows]),
            "iota_c": np.ascontiguousarray(iota_c[rows]),
        })
    return Q, in_maps


LAST_RESULT = None


def kernel(**inputs):
    global LAST_RESULT
    import os
    from concourse.bass_utils import run_bass_kernel_spmd

    Q, in_maps = _host_prep(inputs)
    stage = int(os.environ.get("KERNEL_STAGE", "10"))
    key = (Q, stage)
    if key not in _CACHE:
        _CACHE[key] = _build(Q, stage)
    nc = _CACHE[key]

    res = run_bass_kernel_spmd(nc, in_maps, core_ids=list(range(N_CORES)))
    LAST_RESULT = res

    scores = np.concatenate([res.results[r]["out_scores"] for r in range(N_CORES)], 0)
    preds = np.concatenate([res.results[r]["out_pred"] for r in range(N_CORES)], 0)
    nll = np.concatenate([res.results[r]["out_nll"] for r in range(N_CORES)], 0)
    loss = np.float32(nll[:, 0].mean(dtype=np.float64)
                      + nll[:, 1].mean(dtype=np.float64))
    preds = preds[:, 0].astype(np.int32)
    return loss, scores, preds


# revision 13
# speedup vs baseline: 1.0283x; 1.0283x over previous
"""Trainium2 Bass kernel for nn_DecisionActionAuxiliaryHeads.

Distribution (8 NeuronCores, tensor-parallel):
  - W1/W2 column-sharded (256 cols/core); hidden activations AllGather'd
    between the two MLP layers (transposed layout so no on-device transposes).
  - Warg tensor-sharded over the argument dim (16 args/core). Wname is folded
    into every Warg column on the host (softmax shift-invariance makes the
    pollution cancel in the arg head), so one fused matmul produces
    name+arg+bias scores for all 64*16 (name, local-arg) columns per core.
  - All matmuls run as 3-pass bf16 splits (hi*hi + hi*lo + lo*hi) which is
    ~4e-6 relative accurate - well inside the 1e-3 min top-2 score gap -
    at a fraction of the fp32 LOW_HIGH matmul cost.
  - Candidate / target-arg / name-head values are extracted with segmented
    masked reductions on the Vector engine and scattered into a per-core
    partial tensor, which a ReduceScatter sums and splits over the batch.
  - Loss tail (masked log-softmax NLLs), score masking and argmax run
    per-core on the 16 owned batch rows.

Host side only shards/stages data (slicing, transposes, hi/lo splitting,
one-hot/index encoding); all FLOPs of the model run on the NeuronCores.
"""

import numpy as np

N_CORES = 8
B, S, H, P = 128, 512, 2048, 2048
NN, NA, C = 64, 128, 512
BS = B // N_CORES          # batch rows per core after ReduceScatter
ALS = NA // N_CORES        # args per core
JCOLS = NN * ALS           # 1024 fused (name, local-arg) columns per core
SEG = 8                    # segment width for the candidate gather
NSEG = JCOLS // SEG        # 128
LOGIT_FLOOR = -1e9

SCOL_ARG = C
SCOL_NAME = C + NA
SWIDTH = C + NA + NN       # 704

_CACHE = {}


def _build(Q, stage=8):
    import concourse.bacc as bacc
    import concourse.tile as tile
    import concourse.mybir as mybir

    f32 = mybir.dt.float32
    bf16 = mybir.dt.bfloat16
    i16 = mybir.dt.int16
    u16 = mybir.dt.uint16
    u32 = mybir.dt.uint32
    u8 = mybir.dt.uint8
    Alu = mybir.AluOpType
    Act = mybir.ActivationFunctionType
    AxX = mybir.AxisListType.X

    GW = NSEG * Q * SEG            # candidate product elements per row
    NG = NSEG * Q                  # candidate gather slots per row
    GTOT = NG + ALS + N_CORES      # + arg-head block + name-head block
    MS = P // N_CORES              # 256 MLP cols per core
    KT = H // 128                  # 16 contraction tiles
    NW = JCOLS + N_CORES           # 1032 head columns per core

    nc = bacc.Bacc("TRN2", target_bir_lowering=False, debug=False,
                   num_devices=N_CORES)

    din = lambda name, shape, dt: nc.dram_tensor(name, shape, dt, kind="ExternalInput")
    dout = lambda name, shape, dt: nc.dram_tensor(name, shape, dt, kind="ExternalOutput")

    # hi/lo packed bf16 inputs (cols doubled: [hi | lo])
    pooledhl = din("pooledhl", [H, 2 * B], bf16)
    w1hl = din("w1hl", [H, 2 * MS], bf16)
    w2hl = din("w2hl", [P, 2 * MS], bf16)
    waughl = din("waughl", [P + 2, 2 * NW], bf16)   # rows P/P+1 = bias hi/lo
    adaptT = din("adaptT", [MS, B], f32)
    ascale = din("ascale", [B, 1], f32)
    cmask = din("cmask", [B, GW], bf16)
    sidx = din("sidx", [B, 2 * GTOT], i16)
    ohtid = din("ohtid", [B, NN], f32)
    meff_s = din("meff_s", [BS, C], f32)
    floor_s = din("floor_s", [BS, C], f32)
    meff_a = din("meff_a", [BS, NA], f32)
    floor_a = din("floor_a", [BS, NA], f32)
    meff_n = din("meff_n", [BS, NN], f32)
    floor_n = din("floor_n", [BS, NN], f32)
    oh_ta = din("oh_ta", [BS, NA], f32)
    oh_tn = din("oh_tn", [BS, NN], f32)
    iota_c = din("iota_c", [BS, C], f32)

    out_scores = dout("out_scores", [BS, C], f32)
    out_pred = dout("out_pred", [BS, 1], u32)
    out_nll = dout("out_nll", [BS, 2], f32)

    RG = [list(range(N_CORES))]

    with tile.TileContext(nc) as tc:
        with (
            tc.tile_pool(name="wk", bufs=4) as wk,
            tc.tile_pool(name="acts", bufs=2) as acts,
            tc.tile_pool(name="persist", bufs=1) as persist,
            tc.tile_pool(name="psum", bufs=2, space="PSUM") as psum,
            tc.tile_pool(name="psbig", bufs=1, space="PSUM") as psbig,
            tc.tile_pool(name="dram", bufs=1, space="DRAM") as dram,
            tc.tile_pool(name="tail", bufs=1) as tailp,
        ):
            # ---- collective warmup: pay ncfw/NCCL first-call cost early ----
            wu_in = dram.tile([1, 1], f32)
            wu_out = dram.tile([N_CORES, 1], f32)
            wz = persist.tile([1, 1], f32, tag="wz")
            nc.vector.memset(wz[:], 0.0)
            nc.gpsimd.dma_start(wu_in[:], wz[:])
            nc.gpsimd.collective_compute(
                "AllGather", Alu.bypass, replica_groups=RG,
                ins=[wu_in.opt()], outs=[wu_out.opt()])

            # ---- preload pooled hi/lo (single strided DMA) ----
            pthl = persist.tile([128, KT * 2 * B], bf16, tag="pthl")
            nc.scalar.dma_start(
                pthl[:].rearrange("p (k c) -> p k c", c=2 * B),
                pooledhl[:, :].rearrange("(k p) c -> p k c", p=128))

            def mm3(ps, w_hi, w_lo, x_hi, x_lo, start, stop):
                """psum += W.T @ X with bf16 3-pass split (hi*hi+hi*lo+lo*hi)."""
                nc.tensor.matmul(ps, w_hi, x_hi, start=start, stop=False)
                nc.tensor.matmul(ps, w_hi, x_lo, start=False, stop=False)
                nc.tensor.matmul(ps, w_lo, x_hi, start=False, stop=stop)

            # ---- MLP layer 1: h1T shard = silu(W1_sh^T @ pooled) ----
            h1_bounce = dram.tile([MS, 2 * B], bf16)
            for mt in range(2):
                ps = psum.tile([128, B], f32, space="PSUM", tag="zps")
                for kt in range(KT):
                    lw = wk.tile([128, 2 * MS], bf16, tag="w1t")
                    nc.sync.dma_start(lw[:], w1hl[kt * 128:(kt + 1) * 128, :])
                    c0 = mt * 128
                    mm3(ps[:],
                        lw[:, c0:c0 + 128], lw[:, MS + c0:MS + c0 + 128],
                        pthl[:, kt * 2 * B:kt * 2 * B + B],
                        pthl[:, kt * 2 * B + B:(kt + 1) * 2 * B],
                        start=(kt == 0), stop=(kt == KT - 1))
                sg = acts.tile([128, B], f32, tag="h1sg")
                nc.scalar.activation(sg[:], ps[:], Act.Sigmoid)
                hhl = acts.tile([128, 2 * B], bf16, tag="h1hl")
                hf = acts.tile([128, B], f32, tag="h1f")
                nc.vector.tensor_tensor(hf[:], ps[:], sg[:], op=Alu.mult)
                nc.vector.tensor_copy(hhl[:, 0:B], hf[:])
                nc.vector.tensor_tensor(hhl[:, B:2 * B], hf[:], hhl[:, 0:B],
                                        op=Alu.subtract)
                nc.scalar.dma_start(h1_bounce[mt * 128:(mt + 1) * 128, :], hhl[:])

            ag_h1 = dram.tile([P, 2 * B], bf16)
            nc.gpsimd.collective_compute(
                "AllGather", Alu.bypass, replica_groups=RG,
                ins=[h1_bounce.opt()], outs=[ag_h1.opt()])

            # ---- MLP layer 2: featsT shard = silu(W2_sh^T @ h1) + adapter ----
            h1t = persist.tile([128, KT * 2 * B], bf16, tag="h1t")
            nc.scalar.dma_start(
                h1t[:].rearrange("p (k c) -> p k c", c=2 * B),
                ag_h1[:, :].rearrange("(k p) c -> p k c", p=128))

            asc = persist.tile([B, 1], f32, tag="ascale")
            nc.scalar.dma_start(asc[:], ascale[:, :])

            f_bounce = dram.tile([MS, 2 * B], bf16)
            for mt in range(2):
                ps = psum.tile([128, B], f32, space="PSUM", tag="zps")
                for kt in range(KT):
                    lw = wk.tile([128, 2 * MS], bf16, tag="w2t")
                    nc.sync.dma_start(lw[:], w2hl[kt * 128:(kt + 1) * 128, :])
                    c0 = mt * 128
                    mm3(ps[:],
                        lw[:, c0:c0 + 128], lw[:, MS + c0:MS + c0 + 128],
                        h1t[:, kt * 2 * B:kt * 2 * B + B],
                        h1t[:, kt * 2 * B + B:(kt + 1) * 2 * B],
                        start=(kt == 0), stop=(kt == KT - 1))
                fsg = acts.tile([128, B], f32, tag="fsg")
                nc.scalar.activation(fsg[:], ps[:], Act.Sigmoid)
                f = acts.tile([128, B], f32, tag="ft")
                nc.vector.tensor_tensor(f[:], ps[:], fsg[:], op=Alu.mult)
                at = acts.tile([128, B], f32, tag="adt")
                nc.scalar.dma_start(at[:], adaptT[mt * 128:(mt + 1) * 128, :])
                f2 = acts.tile([128, B], f32, tag="ft2")
                nc.vector.scalar_tensor_tensor(
                    f2[:], at[:], asc[:], f[:], Alu.mult, Alu.add)
                fhl = acts.tile([128, 2 * B], bf16, tag="fhl")
                nc.vector.tensor_copy(fhl[:, 0:B], f2[:])
                nc.vector.tensor_tensor(fhl[:, B:2 * B], f2[:], fhl[:, 0:B],
                                        op=Alu.subtract)
                nc.scalar.dma_start(f_bounce[mt * 128:(mt + 1) * 128, :], fhl[:])

            ag_f = dram.tile([P, 2 * B], bf16)
            nc.gpsimd.collective_compute(
                "AllGather", Alu.bypass, replica_groups=RG,
                ins=[f_bounce.opt()], outs=[ag_f.opt()])

            ft = persist.tile([128, KT * 2 * B], bf16, tag="ft_all")
            nc.scalar.dma_start(
                ft[:].rearrange("p (k c) -> p k c", c=2 * B),
                ag_f[:, :].rearrange("(k p) c -> p k c", p=128))

            # preload head weights + gather inputs early (overlap with MLP)
            wt = []
            for kt in range(KT):
                t = persist.tile([128, 2 * NW], bf16, tag=f"wg{kt}")
                nc.sync.dma_start(t[:], waughl[kt * 128:(kt + 1) * 128, :])
                wt.append(t)
            wb = persist.tile([2, 2 * NW], bf16, tag="waug_b")
            nc.sync.dma_start(wb[:], waughl[P:P + 2, :])
            ones2 = persist.tile([2, 128], bf16, tag="ones2")
            nc.vector.memset(ones2[:], 1.0)

            cm = persist.tile([B, GW], bf16, tag="cmask")
            nc.scalar.dma_start(cm[:], cmask[:, :])
            sx = persist.tile([B, 2 * GTOT], i16, tag="sidx")
            nc.scalar.dma_start(sx[:], sidx[:, :])
            oht = persist.tile([B, NN], f32, tag="ohtid")
            nc.scalar.dma_start(oht[:], ohtid[:, :])

            # ---- fused head matmul, chunk A (cols 0:512) then B (512:1024) ----
            pA = psbig.tile([128, 512], f32, space="PSUM", tag="pA")
            pB = psbig.tile([128, 512], f32, space="PSUM", tag="pB")
            pN = psbig.tile([128, N_CORES], f32, space="PSUM", tag="pN")
            allb = persist.tile([B, JCOLS], f32, tag="allb")
            gath = persist.tile([B, GTOT], f32, tag="gath")
            prod = persist.tile([B, GW // 2], f32, tag="prod")

            def head_pass(pchunk, lo_off, width, extra=()):
                """3-pass accumulate over all k-tiles for one column chunk."""
                for kt in range(KT):
                    fhi = ft[:, kt * 2 * B:kt * 2 * B + B]
                    flo = ft[:, kt * 2 * B + B:(kt + 1) * 2 * B]
                    whi = wt[kt][:, lo_off:lo_off + width]
                    wlo = wt[kt][:, NW + lo_off:NW + lo_off + width]
                    st = (kt == 0)
                    nc.tensor.matmul(pchunk, fhi, whi, start=st, stop=False)
                    nc.tensor.matmul(pchunk, fhi, wlo, start=False, stop=False)
                    nc.tensor.matmul(pchunk, flo, whi, start=False, stop=False)
                    for (px, off2, w2_) in extra:
                        nc.tensor.matmul(px, fhi, wt[kt][:, off2:off2 + w2_],
                                         start=st, stop=False)
                        nc.tensor.matmul(px, fhi,
                                         wt[kt][:, NW + off2:NW + off2 + w2_],
                                         start=False, stop=False)
                        nc.tensor.matmul(px, flo, wt[kt][:, off2:off2 + w2_],
                                         start=False, stop=False)
                # bias rows (k=2 ones matmul adds bias_hi + bias_lo)
                nc.tensor.matmul(pchunk, ones2[:], wb[:, lo_off:lo_off + width],
                                 start=False, stop=True)
                for (px, off2, w2_) in extra:
                    nc.tensor.matmul(px, ones2[:], wb[:, off2:off2 + w2_],
                                     start=False, stop=True)

            # chunk A + name head
            head_pass(pA[:], 0, 512, extra=[(pN[:], JCOLS, N_CORES)])

            # gather for chunk A (reads PSUM directly) while B accumulates
            HSEG = NSEG // 2  # 64 segments per half
            HNG = HSEG * Q

            def gather_half(pchunk, half):
                src = (pchunk.rearrange("p (s t) -> p s t", t=SEG)
                       .unsqueeze(2).to_broadcast([B, HSEG, Q, SEG]))
                cmh = cm[:, half * (GW // 2):(half + 1) * (GW // 2)] \
                    .rearrange("p (s q t) -> p s q t", q=Q, t=SEG)
                ph = prod[:].rearrange("p (s q t) -> p s q t", q=Q, t=SEG)
                nc.vector.tensor_tensor(ph, src, cmh, op=Alu.mult)
                nc.vector.tensor_reduce(
                    gath[:, half * HNG:(half + 1) * HNG],
                    prod[:].rearrange("p (g t) -> p g t", t=SEG),
                    axis=AxX, op=Alu.add)

            gather_half(pA[:], 0)
            nc.vector.tensor_copy(allb[:, 0:512], pA[:])

            # chunk B
            head_pass(pB[:], 512, 512)
            gather_half(pB[:], 1)
            nc.vector.tensor_copy(allb[:, 512:1024], pB[:])

            # arg head: gath[:, NG+q] = sum_n allb[b, n*ALS+q] * ohtid[b, n]
            aprod = persist.tile([B, ALS * NN], f32, tag="aprod")
            allb_a = allb[:].rearrange("p (n q) -> p q n", q=ALS)
            oht_v = oht[:].unsqueeze(1).to_broadcast([B, ALS, NN])
            aprod_v = aprod[:].rearrange("p (q n) -> p q n", n=NN)
            nc.vector.tensor_tensor(aprod_v, allb_a, oht_v, op=Alu.mult)
            nc.vector.tensor_reduce(
                gath[:, NG:NG + ALS],
                aprod[:].rearrange("p (q n) -> p q n", n=NN),
                axis=AxX, op=Alu.add)

            nc.vector.tensor_copy(gath[:, NG + ALS:GTOT], pN[:])

            # ---- scatter into partial-sum tensor ----
            stile = persist.tile([B, SWIDTH], f32, tag="stile")
            nc.vector.memset(stile[:], 0.0)
            nc.gpsimd.local_scatter(
                stile[:].bitcast(u16), gath[:].bitcast(u16), sx[:],
                channels=B, num_elems=2 * SWIDTH, num_idxs=2 * GTOT)

            # ---- ReduceScatter over cores / batch rows ----
            s_bounce = dram.tile([B, SWIDTH], f32)
            nc.scalar.dma_start(s_bounce[:], stile[:])
            rs_out = dram.tile([BS, SWIDTH], f32)
            nc.gpsimd.collective_compute(
                "ReduceScatter", Alu.add, replica_groups=RG,
                ins=[s_bounce.opt()], outs=[rs_out.opt()])
            srs = tailp.tile([BS, SWIDTH], f32, tag="srs")
            nc.scalar.dma_start(srs[:], rs_out[:, :])

            # ---- tails on the 16 owned rows ----
            def load(src, shape, tag):
                t = tailp.tile(shape, f32, tag=tag)
                nc.scalar.dma_start(t[:], src[:, :])
                return t

            ms = load(meff_s, [BS, C], "meff_s")
            fs = load(floor_s, [BS, C], "floor_s")
            sc_m = tailp.tile([BS, C], f32, tag="sc_m")
            nc.vector.tensor_tensor(sc_m[:], srs[:, 0:C], ms[:], op=Alu.mult)
            nc.vector.tensor_tensor(sc_m[:], sc_m[:], fs[:], op=Alu.add)
            nc.scalar.dma_start(out_scores[:, :], sc_m[:])

            if stage >= 7:
                mx = tailp.tile([BS, 1], f32, tag="mx")
                nc.vector.tensor_reduce(mx[:], sc_m[:], axis=AxX, op=Alu.max)
                eq = tailp.tile([BS, C], u8, tag="eq")
                nc.vector.tensor_tensor(eq[:], sc_m[:], mx[:].to_broadcast([BS, C]),
                                        op=Alu.is_equal)
                io = load(iota_c, [BS, C], "iota_c")
                idxm = tailp.tile([BS, C], f32, tag="idxm")
                nc.vector.memset(idxm[:], 1.0e9)
                nc.vector.copy_predicated(idxm[:], eq[:], io[:])
                predf = tailp.tile([BS, 1], f32, tag="predf")
                nc.vector.tensor_reduce(predf[:], idxm[:], axis=AxX, op=Alu.min)
                predu = tailp.tile([BS, 1], u32, tag="predu")
                nc.vector.tensor_copy(predu[:], predf[:])
                nc.scalar.dma_start(out_pred[:, :], predu[:])

            if stage >= 8:
                nll = tailp.tile([BS, 2], f32, tag="nll")

                def head(x_ap, W, meff_t, floor_t, oh_t, col, tag):
                    xm = tailp.tile([BS, W], f32, tag=f"xm{tag}")
                    nc.vector.tensor_tensor(xm[:], x_ap, meff_t[:], op=Alu.mult)
                    nc.vector.tensor_tensor(xm[:], xm[:], floor_t[:], op=Alu.add)
                    m1 = tailp.tile([BS, 1], f32, tag=f"m1{tag}")
                    nc.vector.tensor_reduce(m1[:], xm[:], axis=AxX, op=Alu.max)
                    negm = tailp.tile([BS, 1], f32, tag=f"nm{tag}")
                    nc.scalar.mul(negm[:], m1[:], -1.0)
                    e = tailp.tile([BS, W], f32, tag=f"e{tag}")
                    nc.scalar.activation(e[:], xm[:], Act.Exp, bias=negm[:])
                    s1 = tailp.tile([BS, 1], f32, tag=f"s1{tag}")
                    nc.vector.tensor_reduce(s1[:], e[:], axis=AxX, op=Alu.add)
                    ls = tailp.tile([BS, 1], f32, tag=f"ls{tag}")
                    nc.scalar.activation(ls[:], s1[:], Act.Ln)
                    lse = tailp.tile([BS, 1], f32, tag=f"lse{tag}")
                    nc.vector.tensor_tensor(lse[:], m1[:], ls[:], op=Alu.add)
                    scr = tailp.tile([BS, W], f32, tag=f"scr{tag}")
                    nc.vector.tensor_tensor(scr[:], xm[:], oh_t[:], op=Alu.mult)
                    xt = tailp.tile([BS, 1], f32, tag=f"xt{tag}")
                    nc.vector.tensor_reduce(xt[:], scr[:], axis=AxX, op=Alu.add)
                    nc.vector.tensor_tensor(nll[:, col:col + 1], lse[:], xt[:],
                                            op=Alu.subtract)

                ma = load(meff_a, [BS, NA], "meff_a")
                fa = load(floor_a, [BS, NA], "floor_a")
                ota = load(oh_ta, [BS, NA], "oh_ta")
                mn = load(meff_n, [BS, NN], "meff_n")
                fn = load(floor_n, [BS, NN], "floor_n")
                otn = load(oh_tn, [BS, NN], "oh_tn")
                head(srs[:, SCOL_NAME:SCOL_NAME + NN], NN, mn, fn, otn, 0, "n")
                head(srs[:, SCOL_ARG:SCOL_ARG + NA], NA, ma, fa, ota, 1, "a")
                nc.scalar.dma_start(out_nll[:, :], nll[:])

    nc.compile()
    return nc


def _effective_mask(m):
    m = m.astype(bool)
    any_row = m.any(axis=1, keepdims=True)
    return np.where(any_row, m, True).astype(np.float32)


def _hl(x, bf16):
    """Split fp32 array into packed [hi | lo] bf16 along the last axis."""
    hi = x.astype(bf16)
    lo = (x - hi.astype(np.float32)).astype(bf16)
    return np.concatenate([hi, lo], axis=-1)


def _host_prep(inputs):
    import ml_dtypes
    bf16 = ml_dtypes.bfloat16

    hs = np.asarray(inputs["hidden_states"], np.float32)
    am = np.asarray(inputs["attention_mask"])
    lengths = np.clip(am.astype(np.int64).sum(1), 1, None) - 1
    pooled = hs[np.arange(hs.shape[0]), lengths]
    pooledT = np.ascontiguousarray(pooled.T)          # [P, B]
    pooledhl = _hl(pooledT, bf16)                     # [P, 2B]

    W1 = np.asarray(inputs["W1"], np.float32)
    W2 = np.asarray(inputs["W2"], np.float32)
    adapter_emb = np.asarray(inputs["adapter_emb"], np.float32)
    adapter_scale = np.float32(np.asarray(inputs["adapter_scale"]))
    Wname = np.asarray(inputs["Wname"], np.float32)
    bname = np.asarray(inputs["bname"], np.float32)
    Warg = np.asarray(inputs["Warg"], np.float32)
    barg = np.asarray(inputs["barg"], np.float32)

    adapt = adapter_emb[np.asarray(inputs["adapter_ids"]).astype(np.int64)]
    adaptT = np.ascontiguousarray(adapt.T)            # [P, B]

    cn = np.asarray(inputs["full_candidate_name_ids"]).astype(np.int64)
    ca = np.asarray(inputs["full_candidate_argument_ids"]).astype(np.int64)
    fcm = np.asarray(inputs["full_candidate_masks"])
    tid = np.asarray(inputs["target_name_ids"]).astype(np.int64)
    ta = np.asarray(inputs["target_argument_ids"]).astype(np.int64)
    ncm = np.asarray(inputs["name_candidate_masks"])
    acm = np.asarray(inputs["target_argument_candidate_masks"])

    Q = 1
    per_core = []
    for r in range(N_CORES):
        own = (ca >= ALS * r) & (ca < ALS * (r + 1))
        j = cn * ALS + (ca - ALS * r)
        per_core.append((own, j))
        for b in range(B):
            js = j[b][own[b]]
            if len(js):
                Q = max(Q, int(np.bincount(js // SEG, minlength=NSEG).max()))
    Q = max(Q, 6)

    NG = NSEG * Q
    GTOT = NG + ALS + N_CORES

    meff_s = _effective_mask(fcm)
    floor_s = (1.0 - meff_s) * np.float32(LOGIT_FLOOR)
    meff_n = _effective_mask(ncm)
    floor_n = (1.0 - meff_n) * np.float32(LOGIT_FLOOR)
    meff_a = _effective_mask(acm)
    floor_a = (1.0 - meff_a) * np.float32(LOGIT_FLOOR)
    oh_tn = np.zeros((B, NN), np.float32)
    oh_tn[np.arange(B), tid] = 1.0
    oh_ta = np.zeros((B, NA), np.float32)
    oh_ta[np.arange(B), ta] = 1.0
    iota_c = np.broadcast_to(np.arange(C, dtype=np.float32), (B, C)).copy()

    MS = P // N_CORES
    NW = JCOLS + N_CORES
    in_maps = []
    for r in range(N_CORES):
        own, j = per_core[r]
        cmask = np.zeros((B, NSEG, Q, SEG), np.float32)
        sidx = np.full((B, 2 * GTOT), -1, np.int16)
        for b in range(B):
            cnt = np.zeros(NSEG, np.int64)
            for c in np.flatnonzero(own[b]):
                jj = int(j[b, c])
                s, t = jj // SEG, jj % SEG
                q = cnt[s]
                cnt[s] = q + 1
                cmask[b, s, q, t] = 1.0
                g = s * Q + q
                sidx[b, 2 * g] = 2 * c
                sidx[b, 2 * g + 1] = 2 * c + 1
            for q in range(ALS):
                g = NG + q
                d = SCOL_ARG + ALS * r + q
                sidx[b, 2 * g] = 2 * d
                sidx[b, 2 * g + 1] = 2 * d + 1
            for q in range(N_CORES):
                g = NG + ALS + q
                d = SCOL_NAME + N_CORES * r + q
                sidx[b, 2 * g] = 2 * d
                sidx[b, 2 * g + 1] = 2 * d + 1

        wall = np.ascontiguousarray(
            Warg[:, :, ALS * r:ALS * (r + 1)].transpose(1, 0, 2))  # [P, NN, ALS]
        wall = wall + Wname[:, :, None]
        biasrow = (barg[:, ALS * r:ALS * (r + 1)] + bname[:, None]).reshape(1, -1)
        waug = np.empty((P, NW), np.float32)
        waug[:, :JCOLS] = wall.reshape(P, JCOLS)
        waug[:, JCOLS:] = Wname[:, N_CORES * r:N_CORES * (r + 1)]
        bias = np.zeros((2, NW), np.float32)
        bias[0, :JCOLS] = biasrow
        bias[0, JCOLS:] = bname[N_CORES * r:N_CORES * (r + 1)]
        # bias row split: row0=hi, row1=lo (fp32 bias = hi + lo exactly)
        bias_hi = bias[0].astype(bf16)
        bias_lo = (bias[0] - bias_hi.astype(np.float32)).astype(bf16)
        waughl = np.empty((P + 2, 2 * NW), bf16)
        waughl[:P] = _hl(waug, bf16)
        waughl[P, :NW] = bias_hi
        waughl[P, NW:] = 0.0
        waughl[P + 1, :NW] = bias_lo
        waughl[P + 1, NW:] = 0.0

        rows = slice(BS * r, BS * (r + 1))
        in_maps.append({
            "pooledhl": pooledhl,
            "w1hl": _hl(W1[:, MS * r:MS * (r + 1)], bf16),
            "w2hl": _hl(W2[:, MS * r:MS * (r + 1)], bf16),
            "waughl": waughl,
            "adaptT": np.ascontiguousarray(adaptT[MS * r:MS * (r + 1), :]),
            "ascale": np.full((B, 1), adapter_scale, np.float32),
            "cmask": cmask.reshape(B, -1).astype(bf16),
            "sidx": sidx,
            "ohtid": oh_tn,
            "meff_s": np.ascontiguousarray(meff_s[rows]),
            "floor_s": np.ascontiguousarray(floor_s[rows]),
            "meff_a": np.ascontiguousarray(meff_a[rows]),
            "floor_a": np.ascontiguousarray(floor_a[rows]),
            "meff_n": np.ascontiguousarray(meff_n[rows]),
            "floor_n": np.ascontiguousarray(floor_n[rows]),
            "oh_ta": np.ascontiguousarray(oh_ta[rows]),
            "oh_tn": np.ascontiguousarray(oh_tn[rows]),
            "iota_c": np.ascontiguousarray(iota_c[rows]),
        })
    return Q, in_maps


LAST_RESULT = None


def kernel(**inputs):
    global LAST_RESULT
    import os
    from concourse.bass_utils import run_bass_kernel_spmd

    Q, in_maps = _host_prep(inputs)
    stage = int(os.environ.get("KERNEL_STAGE", "8"))
    key = (Q, stage)
    if key not in _CACHE:
        _CACHE[key] = _build(Q, stage)
    nc = _CACHE[key]

    res = run_bass_kernel_spmd(nc, in_maps, core_ids=list(range(N_CORES)))
    LAST_RESULT = res

    scores = np.concatenate([res.results[r]["out_scores"] for r in range(N_CORES)], 0)
    preds = np.concatenate([res.results[r]["out_pred"] for r in range(N_CORES)], 0)
    nll = np.concatenate([res.results[r]["out_nll"] for r in range(N_CORES)], 0)
    loss = np.float32(nll[:, 0].mean(dtype=np.float64)
                      + nll[:, 1].mean(dtype=np.float64))
    preds = preds[:, 0].astype(np.int32)
    return loss, scores, preds
